# revision 5
# baseline (speedup 1.0000x reference)
"""Trainium2 Bass kernel for AtomicNumberPooling (segment-sum pooling).

Math: output[b, (z-1)*F + f] = sum_{n: batch[n]==b, z[n]==z} out[n, f],
i.e. a segment sum over combined id seg = batch*100 + (z-1), reshaped to
[B, 100*F].

Strategy
--------
`batch` is sorted, so sharding the B=1000 graphs contiguously over 8 cores
gives each core a contiguous row range of `out` and a fully disjoint slice
of the output - no collective needed.

Host-side packing (index bookkeeping + memory movement only, no FLOPs):
  * rows are grouped per graph and zero-padded to GPAD=256 rows/graph
    (the real max is 252 for this input size; overflow rows fall back to a
    host-side correction);
  * each f32 row is split into bf16 hi + lo halves (hi||lo = 512B, the
    same bytes as the f32 row, and hi+lo reconstructs x to ~1e-7);
  * the layout is partition-major ([128, NT*256] bf16) so the device loads
    arrive in a dozen multi-tile DMAs with 12.8KB-contiguous descriptors
    (~425 GB/s instead of the 512B-descriptor penalty rate).

Device program (per core, identical SPMD):
  * DVE builds one-hot(z) [128, 100] tiles via iota + is_equal;
  * PE runs two bf16 matmuls per 128-row tile (hi, lo) accumulating into
    the same PSUM f32 columns - psum[zcol, f] = exact segment sum of one
    graph; 4 graphs share one full PSUM bank (2 banks ping-pong);
  * ACT copies each full bank [100, 4F] to a wide SBUF buffer and DMAs
    20-graph chunks straight to the output slice (ACT has its own HW DGE
    ring, so loads and stores ride separate rings);
  * raw bass Block with explicit single-semaphore waits - the walrus build
    in this container rejects instructions carrying more than one sem wait,
    which rules out Tile-generated IR.

Modeled (CoreSim cost model) exec: ~63.5us/core, which is ~100% of the
padded-layout HBM roofline (16.4MB in + 6.4MB out at ~360GB/s shared).
"""

from contextlib import ExitStack

import ml_dtypes
import numpy as np

import concourse.bass as bass
import concourse.mybir as mybir
from concourse.bass_utils import run_bass_kernel_spmd

NCORES = 8
B = 1000
MAX_Z = 100
F = 128
TP = 128                 # rows per tile (SBUF partition dim)
GB = B // NCORES         # graphs per core
GPAD = 256               # padded rows per graph (real max for this input: 252)
SPG = GPAD // TP         # row tiles per graph
NT = GB * SPG            # row tiles per core
RPC = GB * GPAD          # padded rows per core
BF16 = ml_dtypes.bfloat16


def _load_chunks(nt):
    """Graduated load-chunk sizes (in tiles): small first chunks so compute
    starts early, 25-tile (1.6MB) steady state."""
    sizes = []
    for s in (4, 8, 16):
        if sum(sizes) + s <= nt:
            sizes.append(s)
        else:
            break
    while sum(sizes) + 25 <= nt:
        sizes.append(25)
    rem = nt - sum(sizes)
    if rem > 0:
        sizes.append(rem)
    return sizes


def _build(gb=GB, gpad=GPAD, ohbuf=16, gchunk=20, start_clear=True):
    spg = gpad // TP
    nt = gb * spg
    F2 = 2 * F
    chunks = _load_chunks(nt)
    nch = len(chunks)
    starts = [sum(chunks[:i]) for i in range(nch)]
    assert gchunk % 4 == 0
    nsc_full, tail = divmod(gb, gchunk)
    store_chunks = [gchunk] * nsc_full + ([tail] if tail else [])
    nsc = len(store_chunks)
    ngrp = (gb + 3) // 4                      # psum groups of 4 graphs

    nc = bass.Bass()
    xp = nc.dram_tensor("xp", [TP, nt * F2], mybir.dt.bfloat16, kind="ExternalInput")
    zc = nc.dram_tensor("zc", [TP, nt], mybir.dt.float32, kind="ExternalInput")
    o = nc.dram_tensor("o", [gb * MAX_Z, F], mybir.dt.float32, kind="ExternalOutput")

    with ExitStack() as ctx:
        iota = ctx.enter_context(nc.sbuf_tensor("iota", [TP, MAX_Z], mybir.dt.float32))
        zcs = ctx.enter_context(nc.sbuf_tensor("zcs", [TP, nt], mybir.dt.float32))
        xbig = ctx.enter_context(
            nc.sbuf_tensor("xbig", [TP, nt * F2], mybir.dt.bfloat16))
        oh = [
            ctx.enter_context(nc.sbuf_tensor(f"oh{i}", [TP, MAX_Z], mybir.dt.bfloat16))
            for i in range(ohbuf)
        ]
        psum = [
            ctx.enter_context(nc.psum_tensor(f"ps{i}", [MAX_Z, 4 * F],
                                             mybir.dt.float32))
            for i in range(2)
        ]
        osb = [
            ctx.enter_context(
                nc.sbuf_tensor(f"osb{i}", [MAX_Z, gchunk * F], mybir.dt.float32))
            for i in range(2)
        ]

        s_const = ctx.enter_context(nc.semaphore("s_const"))
        s_iota = ctx.enter_context(nc.semaphore("s_iota"))
        s_load = [ctx.enter_context(nc.semaphore(f"s_load{i}")) for i in range(nch)]
        s_oh = ctx.enter_context(nc.semaphore("s_oh"))
        s_mm = ctx.enter_context(nc.semaphore("s_mm"))
        s_cp = ctx.enter_context(nc.semaphore("s_cp"))     # +1 per group copy
        s_ost = [ctx.enter_context(nc.semaphore(f"s_ost{i}")) for i in range(2)]
        my_sems = [s_const, s_iota, *s_load, s_oh, s_mm, s_cp, *s_ost]

        if start_clear:
            nc.gpsimd.dma_reset()
            for s in my_sems:
                nc.gpsimd.sem_clear(s)
            nc._nrt_pseudo_barrier()

        with nc.Block() as block:

            @block.sync
            def _(sync):
                sync.dma_start(zcs[:], zc[:]).then_inc(s_const, 16)
                for c in range(nch):
                    c0, w = starts[c], chunks[c]
                    sync.dma_start(
                        xbig[:, c0 * F2:(c0 + w) * F2],
                        xp[:, c0 * F2:(c0 + w) * F2],
                    ).then_inc(s_load[c], 16)

            @block.scalar
            def _(scalar):
                for q in range(ngrp):
                    g0 = 4 * q
                    gw = min(4, gb - g0)              # graphs in this group
                    sc = g0 // gchunk                 # store chunk (aligned)
                    jq = g0 - sc * gchunk             # col offset in osb (graphs)
                    g_last = g0 + gw - 1
                    scalar.wait_ge(s_mm, spg * (g_last + 1))
                    if jq < 4 and sc >= 2:
                        # osb buffer free once its previous store completed
                        scalar.wait_ge(s_ost[sc % 2], 16 * (sc // 2))
                    scalar.copy(
                        osb[sc % 2][:, jq * F:(jq + gw) * F],
                        psum[q % 2][:, 0:gw * F],
                    ).then_inc(s_cp, 1)
                    if g_last == sc * gchunk + store_chunks[sc] - 1:
                        # chunk complete -> store it (ACT HWDGE ring)
                        scalar.wait_ge(s_cp, q + 1)
                        cw = store_chunks[sc]
                        dst = o[sc * gchunk * MAX_Z:
                                (sc * gchunk + cw) * MAX_Z, :].rearrange(
                            "(j p) f -> p j f", p=MAX_Z)
                        src = osb[sc % 2][:, 0:cw * F].rearrange(
                            "p (j f) -> p j f", f=F)
                        scalar.dma_start(dst, src).then_inc(s_ost[sc % 2], 16)

            @block.gpsimd
            def _(gpsimd):
                gpsimd.iota(
                    iota[:], pattern=[[1, MAX_Z]], base=0, channel_multiplier=0,
                    allow_small_or_imprecise_dtypes=True,
                ).then_inc(s_iota, 1)
                for i in range(2):
                    n_stores = len(range(i, nsc, 2))
                    if n_stores:
                        gpsimd.wait_ge(s_ost[i], 16 * n_stores)

            @block.vector
            def _(vector):
                vector.wait_ge(s_const, 16)
                vector.wait_ge(s_iota, 1)
                for t in range(nt):
                    if t >= ohbuf and t % 8 == 0:
                        # frees one-hot slots for tiles t..t+7
                        vector.wait_ge(s_mm, t - 8)
                    vector.tensor_tensor(
                        oh[t % ohbuf][:], iota[:],
                        zcs[:, t:t + 1].broadcast_to([TP, MAX_Z]),
                        op=mybir.AluOpType.is_equal,
                    ).then_inc(s_oh, 1)

            @block.tensor
            def _(tensor):
                cb = 0
                for t in range(nt):
                    g, s = divmod(t, spg)
                    q, gq = divmod(g, 4)
                    if cb < nch and t == starts[cb]:
                        tensor.wait_ge(s_load[cb], 16)
                        cb += 1
                    if t % 4 == 0:
                        tensor.wait_ge(s_oh, min(t + 4, nt))
                    if s == 0 and gq == 0 and q >= 2:
                        tensor.wait_ge(s_cp, q - 1)   # psum bank free
                    tensor.matmul(
                        psum[q % 2][:, gq * F:(gq + 1) * F], oh[t % ohbuf][:],
                        xbig[:, t * F2:t * F2 + F],
                        start=(s == 0), stop=False,
                    )
                    tensor.matmul(
                        psum[q % 2][:, gq * F:(gq + 1) * F], oh[t % ohbuf][:],
                        xbig[:, t * F2 + F:(t + 1) * F2],
                        start=False, stop=(s == spg - 1),
                    ).then_inc(s_mm, 1)

        # Block exit emitted an all-engine barrier: everything is quiesced.
        # Leave sems at zero for the next execution of this NEFF.
        for s in my_sems:
            nc.gpsimd.sem_clear(s)

    return nc


_NC = None


def _get_nc():
    global _NC
    if _NC is None:
        _NC = _build()
    return _NC


def _pack_inputs(x, z, b):
    """Build per-core input maps; returns (in_maps, host_fix).

    host_fix is a [B*MAX_Z, F] float32 correction for rows that could not be
    placed on the device (graph overflow beyond GPAD) - all zeros for sane
    inputs; kept for robustness.
    """
    in_maps = []
    host_fix = None
    zcol = z.astype(np.int64) - 1
    xhi = x.astype(BF16)
    xlo = (x - xhi.astype(np.float32)).astype(BF16)
    for c in range(NCORES):
        g_lo, g_hi = c * GB, (c + 1) * GB
        r0 = np.searchsorted(b, g_lo, side="left")
        r1 = np.searchsorted(b, g_hi, side="left")
        bb = (b[r0:r1] - g_lo).astype(np.int64)
        zz = zcol[r0:r1]
        hh = xhi[r0:r1]
        ll = xlo[r0:r1]

        cnt = np.bincount(bb, minlength=GB)
        offs = np.zeros(GB + 1, np.int64)
        offs[1:] = np.cumsum(cnt)
        rank = np.arange(len(bb)) - offs[bb]

        ok = rank < GPAD
        if not ok.all():
            # overflow rows: accumulate on host (never hit for this dataset)
            if host_fix is None:
                host_fix = np.zeros((B * MAX_Z, F), np.float32)
            sel = ~ok
            good = (zz[sel] >= 0) & (zz[sel] < MAX_Z)
            seg = (b[r0:r1][sel][good].astype(np.int64) * MAX_Z + zz[sel][good])
            np.add.at(host_fix, seg, x[r0:r1][sel][good])
            bb, zz, hh, ll, rank = bb[ok], zz[ok], hh[ok], ll[ok], rank[ok]

        dest = bb * GPAD + rank
        xhl = np.zeros((RPC, 2 * F), BF16)
        xhl[dest, :F] = hh
        xhl[dest, F:] = ll
        # partition-major: row r -> xp[r % 128, (r // 128)*256 : ...]
        xp = np.ascontiguousarray(
            xhl.reshape(NT, TP, 2 * F).transpose(1, 0, 2).reshape(TP, NT * 2 * F))
        zp = np.full(RPC, -1.0, np.float32)   # padding never matches iota
        zp[dest] = zz.astype(np.float32)      # out-of-range z never matches
        zcs = np.ascontiguousarray(zp.reshape(NT, TP).T)
        in_maps.append({"xp": xp, "zc": zcs})
    return in_maps, host_fix


def kernel(out, z, batch):
    x = np.asarray(out, dtype=np.float32)
    z = np.asarray(z)
    b = np.asarray(batch)

    if np.any(b[1:] < b[:-1]):                # robustness: ensure sorted
        order = np.argsort(b, kind="stable")
        x, z, b = x[order], z[order], b[order]
    valid = (b >= 0) & (b < B)                # out-of-range graphs: dropped
    if not valid.all():
        x, z, b = x[valid], z[valid], b[valid]

    in_maps, host_fix = _pack_inputs(x, z, b)
    res = run_bass_kernel_spmd(_get_nc(), in_maps, list(range(NCORES)))
    pooled = np.concatenate([res.results[c]["o"] for c in range(NCORES)], axis=0)
    if host_fix is not None:
        pooled = pooled + host_fix
    return pooled.reshape(B, MAX_Z * F)


# revision 6
# speedup vs baseline: 1.1871x; 1.1871x over previous
"""Trainium2 Bass kernel for AtomicNumberPooling (segment-sum pooling).

Math: output[b, (z-1)*F + f] = sum_{n: batch[n]==b, z[n]==z} out[n, f],
i.e. a segment sum over combined id seg = batch*100 + (z-1), reshaped to
[B, 100*F].

Strategy
--------
`batch` is sorted, so sharding the B=1000 graphs contiguously over 8 cores
gives each core a contiguous row range of `out` and a fully disjoint slice
of the output - no collective needed.

Host-side packing (index bookkeeping + memory movement only, no FLOPs):
  * rows are grouped per graph and zero-padded to GPAD=256 rows/graph
    (the real max is 252 for this input size; overflow rows fall back to a
    host-side correction);
  * each f32 row is split into a bf16 hi half (256B) and an e4m3 fp8 lo
    residual scaled by 2^6 (128B) - 384B/row instead of 512B, with the fp8
    scale folded back inside the matmul (see below); reconstruction error
    is ~5e-5 absmax-relative on the pooled output;
  * layout is partition-major ([128, NT*F]) so device loads arrive in a
    dozen multi-tile DMAs with multi-KB contiguous descriptors.

Device program (per core, identical SPMD):
  * DVE builds one-hot(z) [128, 4*100] tiles (4 row tiles per compare) via
    iota + is_equal; Pool (gpsimd) casts them to e4m3 scaled by 2^-6, so
    MM_lo(onehot*2^-6, lo*2^6) accumulates exactly `lo`;
  * PE runs two matmuls per 128-row tile - bf16 hi and fp8 lo - into the
    same PSUM f32 columns: psum[zcol, f] = segment sum for one graph;
    4 graphs share one full PSUM bank, 4 banks rotate;
  * ACT copies each full bank [100, 4F] into a wide SBUF buffer and DMAs
    12-graph chunks to the output slice on its own HW DGE ring; 4 output
    buffers keep copies running while stores queue behind loads;
  * raw bass Block with explicit single-semaphore waits - the walrus build
    in this container rejects instructions carrying more than one sem wait,
    which rules out Tile-generated IR.

Modeled (CoreSim cost model) exec: ~53.5us/core, ~1us above the HBM floor
for 12.4MB in + 6.4MB out at ~360GB/s shared.
"""

from contextlib import ExitStack

import ml_dtypes
import numpy as np

import concourse.bass as bass
import concourse.mybir as mybir
from concourse.bass_utils import run_bass_kernel_spmd

NCORES = 8
B = 1000
MAX_Z = 100
F = 128
TP = 128                 # rows per tile (SBUF partition dim)
GB = B // NCORES         # graphs per core
GPAD = 256               # padded rows per graph (real max for this input: 252)
SPG = GPAD // TP         # row tiles per graph
NT = GB * SPG            # row tiles per core
RPC = GB * GPAD          # padded rows per core
BF16 = ml_dtypes.bfloat16
E4M3 = ml_dtypes.float8_e4m3
LO_SCALE = 64.0          # host stores lo*2^6; device one-hot carries 2^-6


def _load_chunks(nt):
    """Graduated load-chunk sizes (in tiles): small first chunks so compute
    starts early, 25-tile steady state."""
    sizes = []
    for s in (4, 8, 16):
        if sum(sizes) + s <= nt:
            sizes.append(s)
        else:
            break
    while sum(sizes) + 25 <= nt:
        sizes.append(25)
    rem = nt - sum(sizes)
    if rem > 0:
        sizes.append(rem)
    return sizes


def _build(gb=GB, gpad=GPAD, ohbuf=8, gchunk=12, pbuf=4, osbuf=4,
           start_clear=True):
    spg = gpad // TP
    nt = gb * spg
    bsizes = [4] * (nt // 4) + ([nt % 4] if nt % 4 else [])
    nb = len(bsizes)                          # one-hot batches (mostly 4 tiles)
    bstarts = [sum(bsizes[:i]) for i in range(nb)]
    tile_batch = [0] * nt
    tile_off = [0] * nt
    for i, (b0, w) in enumerate(zip(bstarts, bsizes)):
        for k in range(w):
            tile_batch[b0 + k] = i
            tile_off[b0 + k] = k
    chunks = _load_chunks(nt)
    nch = len(chunks)
    starts = [sum(chunks[:i]) for i in range(nch)]
    assert gchunk % 4 == 0
    nsc_full, tail = divmod(gb, gchunk)
    store_chunks = [gchunk] * nsc_full + ([tail] if tail else [])
    nsc = len(store_chunks)
    ngrp = (gb + 3) // 4                      # psum groups of 4 graphs

    nc = bass.Bass()
    xh = nc.dram_tensor("xh", [TP, nt * F], mybir.dt.bfloat16, kind="ExternalInput")
    xl = nc.dram_tensor("xl", [TP, nt * F], mybir.dt.float8e4, kind="ExternalInput")
    zc = nc.dram_tensor("zc", [TP, nt], mybir.dt.float32, kind="ExternalInput")
    o = nc.dram_tensor("o", [gb * MAX_Z, F], mybir.dt.float32, kind="ExternalOutput")

    with ExitStack() as ctx:
        iota = ctx.enter_context(
            nc.sbuf_tensor("iota", [TP, 4 * MAX_Z], mybir.dt.float32))
        zcs = ctx.enter_context(nc.sbuf_tensor("zcs", [TP, nt], mybir.dt.float32))
        xhb = ctx.enter_context(nc.sbuf_tensor("xhb", [TP, nt * F], mybir.dt.bfloat16))
        xlb = ctx.enter_context(nc.sbuf_tensor("xlb", [TP, nt * F], mybir.dt.float8e4))
        ohb = [
            ctx.enter_context(
                nc.sbuf_tensor(f"ohb{i}", [TP, 4 * MAX_Z], mybir.dt.bfloat16))
            for i in range(ohbuf)
        ]
        ohe = [
            ctx.enter_context(
                nc.sbuf_tensor(f"ohe{i}", [TP, 4 * MAX_Z], mybir.dt.float8e4))
            for i in range(ohbuf)
        ]
        psum = [
            ctx.enter_context(nc.psum_tensor(f"ps{i}", [MAX_Z, 4 * F],
                                             mybir.dt.float32))
            for i in range(pbuf)
        ]
        osb = [
            ctx.enter_context(
                nc.sbuf_tensor(f"osb{i}", [MAX_Z, gchunk * F], mybir.dt.float32))
            for i in range(osbuf)
        ]

        s_const = ctx.enter_context(nc.semaphore("s_const"))
        s_iota = ctx.enter_context(nc.semaphore("s_iota"))
        s_lh = [ctx.enter_context(nc.semaphore(f"s_lh{i}")) for i in range(nch)]
        s_ll = [ctx.enter_context(nc.semaphore(f"s_ll{i}")) for i in range(nch)]
        s_oh = ctx.enter_context(nc.semaphore("s_oh"))     # +1 per DVE batch
        s_oe = ctx.enter_context(nc.semaphore("s_oe"))     # +1 per Pool cast
        s_mm = ctx.enter_context(nc.semaphore("s_mm"))     # +1 per row tile
        s_cp = ctx.enter_context(nc.semaphore("s_cp"))     # +1 per group copy
        s_ost = [ctx.enter_context(nc.semaphore(f"s_ost{i}")) for i in range(osbuf)]
        my_sems = [s_const, s_iota, *s_lh, *s_ll, s_oh, s_oe, s_mm, s_cp, *s_ost]

        if start_clear:
            nc.gpsimd.dma_reset()
            for s in my_sems:
                nc.gpsimd.sem_clear(s)
            nc._nrt_pseudo_barrier()

        with nc.Block() as block:

            @block.sync
            def _(sync):
                sync.dma_start(zcs[:], zc[:]).then_inc(s_const, 16)
                for c in range(nch):
                    c0, w = starts[c], chunks[c]
                    sync.dma_start(
                        xhb[:, c0 * F:(c0 + w) * F],
                        xh[:, c0 * F:(c0 + w) * F],
                    ).then_inc(s_lh[c], 16)
                    sync.dma_start(
                        xlb[:, c0 * F:(c0 + w) * F],
                        xl[:, c0 * F:(c0 + w) * F],
                    ).then_inc(s_ll[c], 16)

            @block.scalar
            def _(scalar):
                for q in range(ngrp):
                    g0 = 4 * q
                    gw = min(4, gb - g0)              # graphs in this group
                    sc = g0 // gchunk                 # store chunk (aligned)
                    jq = g0 - sc * gchunk             # col offset in osb (graphs)
                    g_last = g0 + gw - 1
                    scalar.wait_ge(s_mm, spg * (g_last + 1))
                    if jq < 4 and sc >= osbuf:
                        # osb buffer free once its previous store completed
                        scalar.wait_ge(s_ost[sc % osbuf], 16 * (sc // osbuf))
                    scalar.copy(
                        osb[sc % osbuf][:, jq * F:(jq + gw) * F],
                        psum[q % pbuf][:, 0:gw * F],
                    ).then_inc(s_cp, 1)
                    if g_last == sc * gchunk + store_chunks[sc] - 1:
                        # chunk complete -> store it (ACT HWDGE ring)
                        scalar.wait_ge(s_cp, q + 1)
                        cw = store_chunks[sc]
                        dst = o[sc * gchunk * MAX_Z:
                                (sc * gchunk + cw) * MAX_Z, :].rearrange(
                            "(j p) f -> p j f", p=MAX_Z)
                        src = osb[sc % osbuf][:, 0:cw * F].rearrange(
                            "p (j f) -> p j f", f=F)
                        scalar.dma_start(dst, src).then_inc(s_ost[sc % osbuf], 16)

            @block.vector
            def _(vector):
                vector.wait_ge(s_const, 16)
                vector.wait_ge(s_iota, 1)
                for bq in range(nb):
                    b0, w = bstarts[bq], bsizes[bq]
                    if bq >= ohbuf:
                        # ohb slot free once Pool cast + PE hi-MMs consumed it;
                        # PE implies Pool (MM_lo waits the cast)
                        vector.wait_ge(s_mm, bstarts[bq - ohbuf + 1])
                    vector.tensor_tensor(
                        ohb[bq % ohbuf][:, 0:w * MAX_Z]
                            .rearrange("p (t z) -> p t z", z=MAX_Z),
                        iota[:, 0:w * MAX_Z]
                            .rearrange("p (t z) -> p t z", z=MAX_Z),
                        zcs[:, b0:b0 + w].broadcast_to([TP, w, MAX_Z]),
                        op=mybir.AluOpType.is_equal,
                    ).then_inc(s_oh, 1)

            @block.gpsimd
            def _(gpsimd):
                gpsimd.iota(
                    iota[:], pattern=[[0, 4], [1, MAX_Z]], base=0,
                    channel_multiplier=0,
                    allow_small_or_imprecise_dtypes=True,
                ).then_inc(s_iota, 1)
                for bq in range(nb):
                    w = bsizes[bq]
                    if bq >= ohbuf:
                        gpsimd.wait_ge(s_mm, bstarts[bq - ohbuf + 1])
                    gpsimd.wait_ge(s_oh, bq + 1)
                    gpsimd.tensor_scalar_mul(
                        ohe[bq % ohbuf][:, 0:w * MAX_Z],
                        ohb[bq % ohbuf][:, 0:w * MAX_Z], 1.0 / LO_SCALE,
                    ).then_inc(s_oe, 1)
                for i in range(osbuf):
                    n_stores = len(range(i, nsc, osbuf))
                    if n_stores:
                        gpsimd.wait_ge(s_ost[i], 16 * n_stores)

            @block.tensor
            def _(tensor):
                cb = 0
                for t in range(nt):
                    g, s = divmod(t, spg)
                    q, gq = divmod(g, 4)
                    bq, bt = tile_batch[t], tile_off[t]
                    if cb < nch and t == starts[cb]:
                        tensor.wait_ge(s_lh[cb], 16)
                        tensor.wait_ge(s_ll[cb], 16)
                        cb += 1
                    if bt == 0:
                        tensor.wait_ge(s_oh, bq + 1)
                    if s == 0 and gq == 0 and q >= pbuf:
                        tensor.wait_ge(s_cp, q - pbuf + 1)   # psum bank free
                    tensor.matmul(
                        psum[q % pbuf][:, gq * F:(gq + 1) * F],
                        ohb[bq % ohbuf][:, bt * MAX_Z:(bt + 1) * MAX_Z],
                        xhb[:, t * F:(t + 1) * F],
                        start=(s == 0), stop=False,
                    )
                    if bt == 0:
                        tensor.wait_ge(s_oe, bq + 1)
                    tensor.matmul(
                        psum[q % pbuf][:, gq * F:(gq + 1) * F],
                        ohe[bq % ohbuf][:, bt * MAX_Z:(bt + 1) * MAX_Z],
                        xlb[:, t * F:(t + 1) * F],
                        start=False, stop=(s == spg - 1),
                    ).then_inc(s_mm, 1)

        # Block exit emitted an all-engine barrier: everything is quiesced.
        # Leave sems at zero for the next execution of this NEFF.
        for s in my_sems:
            nc.gpsimd.sem_clear(s)

    return nc


_NC = None


def _get_nc():
    global _NC
    if _NC is None:
        _NC = _build()
    return _NC


def _pack_inputs(x, z, b):
    """Build per-core input maps; returns (in_maps, host_fix).

    host_fix is a [B*MAX_Z, F] float32 correction for rows that could not be
    placed on the device (graph overflow beyond GPAD) - all zeros for sane
    inputs; kept for robustness.
    """
    in_maps = []
    host_fix = None
    zcol = z.astype(np.int64) - 1
    xhi = x.astype(BF16)
    xlo = ((x - xhi.astype(np.float32)) * LO_SCALE).astype(E4M3)
    for c in range(NCORES):
        g_lo, g_hi = c * GB, (c + 1) * GB
        r0 = np.searchsorted(b, g_lo, side="left")
        r1 = np.searchsorted(b, g_hi, side="left")
        bb = (b[r0:r1] - g_lo).astype(np.int64)
        zz = zcol[r0:r1]
        hh = xhi[r0:r1]
        ll = xlo[r0:r1]

        cnt = np.bincount(bb, minlength=GB)
        offs = np.zeros(GB + 1, np.int64)
        offs[1:] = np.cumsum(cnt)
        rank = np.arange(len(bb)) - offs[bb]

        ok = rank < GPAD
        if not ok.all():
            # overflow rows: accumulate on host (never hit for this dataset)
            if host_fix is None:
                host_fix = np.zeros((B * MAX_Z, F), np.float32)
            sel = ~ok
            good = (zz[sel] >= 0) & (zz[sel] < MAX_Z)
            seg = (b[r0:r1][sel][good].astype(np.int64) * MAX_Z + zz[sel][good])
            np.add.at(host_fix, seg, x[r0:r1][sel][good])
            bb, zz, hh, ll, rank = bb[ok], zz[ok], hh[ok], ll[ok], rank[ok]

        dest = bb * GPAD + rank
        xhp = np.zeros((RPC, F), BF16)
        xhp[dest] = hh
        xlp = np.zeros((RPC, F), E4M3)
        xlp[dest] = ll
        # partition-major: row r -> [r % 128, (r // 128)*F : ...]
        xhm = np.ascontiguousarray(
            xhp.reshape(NT, TP, F).transpose(1, 0, 2).reshape(TP, NT * F))
        xlm = np.ascontiguousarray(
            xlp.reshape(NT, TP, F).transpose(1, 0, 2).reshape(TP, NT * F))
        zp = np.full(RPC, -1.0, np.float32)   # padding never matches iota
        zp[dest] = zz.astype(np.float32)      # out-of-range z never matches
        zcs = np.ascontiguousarray(zp.reshape(NT, TP).T)
        in_maps.append({"xh": xhm, "xl": xlm, "zc": zcs})
    return in_maps, host_fix


def kernel(out, z, batch):
    x = np.asarray(out, dtype=np.float32)
    z = np.asarray(z)
    b = np.asarray(batch)

    if np.any(b[1:] < b[:-1]):                # robustness: ensure sorted
        order = np.argsort(b, kind="stable")
        x, z, b = x[order], z[order], b[order]
    valid = (b >= 0) & (b < B)                # out-of-range graphs: dropped
    if not valid.all():
        x, z, b = x[valid], z[valid], b[valid]

    in_maps, host_fix = _pack_inputs(x, z, b)
    res = run_bass_kernel_spmd(_get_nc(), in_maps, list(range(NCORES)))
    pooled = np.concatenate([res.results[c]["o"] for c in range(NCORES)], axis=0)
    if host_fix is not None:
        pooled = pooled + host_fix
    return pooled.reshape(B, MAX_Z * F)
